# revision 1
# baseline (speedup 1.0000x reference)
"""Trainium2 Bass kernel for DAGConstraintLayer: sigmoid + binary-tree min-propagation.

Full input x: (262144, 127) f32. out[b, i] = min over ancestors a of node i
(inclusive, in a complete binary tree parent(i)=(i-1)//2) of sigmoid(x[b, a]).

Sharding: pure data parallelism over the batch dim across 8 NeuronCores.
Per core: (32768, 127). Layout: partition p holds 256 consecutive rows of the
core's slice, so DMAs are large fully-contiguous-per-partition transfers.
Compute per chunk: ACT sigmoid over the whole tile, then one DVE min per tree
level with the parent operand broadcast (stride-0 last axis) over its 2 children.
"""

import sys

sys.path.insert(0, "/opt/trn_rl_repo")

import numpy as np

import concourse.bacc as bacc
import concourse.mybir as mybir
import concourse.tile as tile
from concourse.bass_utils import run_bass_kernel_spmd

BATCH = 262144
NODES = 127
DEPTH = 7
NCORES = 8
B_CORE = BATCH // NCORES          # 32768 rows per core
ROWS_PER_PART = B_CORE // 128     # 256 rows per partition
T = 4                             # chunks per core
G = ROWS_PER_PART // T            # 64 rows per partition per chunk
W = G * NODES                     # 8128 f32 per partition per chunk

_cache = {}


def _build():
    nc = bacc.Bacc("TRN2", target_bir_lowering=False, debug=False)
    x_d = nc.dram_tensor("x", (B_CORE, NODES), mybir.dt.float32, kind="ExternalInput")
    o_d = nc.dram_tensor("out", (B_CORE, NODES), mybir.dt.float32, kind="ExternalOutput")
    xf = x_d[:].rearrange("(p r) d -> p (r d)", p=128)
    of = o_d[:].rearrange("(p r) d -> p (r d)", p=128)

    with tile.TileContext(nc) as tc:
        with (
            tc.tile_pool(name="inp", bufs=2) as inp,
            tc.tile_pool(name="outp", bufs=2) as outp,
        ):
            for t in range(T):
                ti = inp.tile([128, W], mybir.dt.float32)
                nc.sync.dma_start(ti[:], xf[:, t * W : (t + 1) * W])
                to = outp.tile([128, W], mybir.dt.float32)
                nc.scalar.activation(
                    to[:], ti[:], mybir.ActivationFunctionType.Sigmoid
                )
                o3 = to[:].rearrange("p (g d) -> p g d", d=NODES)
                for level in range(1, DEPTH):
                    c = 2 ** (level - 1)          # number of parents
                    s0 = c - 1                    # first parent
                    s1 = 2 * c - 1                # first child
                    ch = o3[:, :, s1 : s1 + 2 * c].rearrange(
                        "p g (c two) -> p g c two", two=2
                    )
                    pa = (
                        o3[:, :, s0 : s0 + c]
                        .unsqueeze(3)
                        .broadcast_to([128, G, c, 2])
                    )
                    nc.vector.tensor_tensor(
                        out=ch, in0=ch, in1=pa, op=mybir.AluOpType.min
                    )
                nc.sync.dma_start(of[:, t * W : (t + 1) * W], to[:])
    nc.compile()
    return nc


def run(x, trace=False):
    x = np.asarray(x, dtype=np.float32)
    assert x.shape == (BATCH, NODES)
    if "nc" not in _cache:
        _cache["nc"] = _build()
    nc = _cache["nc"]
    in_maps = [
        {"x": np.ascontiguousarray(x[c * B_CORE : (c + 1) * B_CORE])}
        for c in range(NCORES)
    ]
    res = run_bass_kernel_spmd(nc, in_maps, list(range(NCORES)), trace=trace)
    out = np.concatenate([res.results[c]["out"] for c in range(NCORES)], axis=0)
    return out, res


def kernel(x):
    out, _ = run(x)
    return out


# revision 5
# speedup vs baseline: 3.5224x; 3.5224x over previous
"""Trainium2 Bass kernel for DAGConstraintLayer: sigmoid + binary-tree min-propagation.

Full input x: (262144, 127) f32. out[b, i] = min over ancestors a of node i
(inclusive, in a complete binary tree parent(i)=(i-1)//2) of sigmoid(x[b, a]).

Sharding: pure data parallelism over the batch dim across 8 NeuronCores.
Per core: (32768, 127). Layout: partition p holds 256 consecutive rows of the
core's slice, so DMAs are large fully-contiguous-per-partition transfers.
Compute per chunk: ACT sigmoid over the whole tile, then one DVE min per tree
level with the parent operand broadcast (stride-0 last axis) over its 2 children.
"""

import os
import sys

for _p in ("/opt/trn_rl_repo", "/root/.axon_site/_ro/trn_rl_repo"):
    if os.path.isdir(_p) and _p not in sys.path:
        sys.path.append(_p)

import numpy as np

import concourse.bacc as bacc
import concourse.mybir as mybir
import concourse.tile as tile
from concourse.bass_utils import run_bass_kernel_spmd

BATCH = 262144
NODES = 127
DEPTH = 7
NCORES = 8
B_CORE = BATCH // NCORES          # 32768 rows per core
ROWS_PER_PART = B_CORE // 128     # 256 rows per partition
T = 8                             # chunks per core
G = ROWS_PER_PART // T            # 32 rows per partition per chunk
W = G * NODES                     # 4064 f32 per partition per chunk
BUFS = 4

_cache = {}


def _build():
    nc = bacc.Bacc("TRN2", target_bir_lowering=False, debug=False)
    x_d = nc.dram_tensor("x", (B_CORE, NODES), mybir.dt.float32, kind="ExternalInput")
    o_d = nc.dram_tensor("out", (B_CORE, NODES), mybir.dt.float32, kind="ExternalOutput")
    xf = x_d[:].rearrange("(p r) d -> p (r d)", p=128)
    of = o_d[:].rearrange("(p r) d -> p (r d)", p=128)

    with tile.TileContext(nc) as tc:
        with (
            tc.tile_pool(name="inp", bufs=BUFS) as inp,
            tc.tile_pool(name="outp", bufs=BUFS) as outp,
        ):
            for t in range(T):
                ti = inp.tile([128, W], mybir.dt.float32)
                nc.sync.dma_start(ti[:], xf[:, t * W : (t + 1) * W])
                to = outp.tile([128, W], mybir.dt.float32)
                nc.scalar.activation(
                    to[:], ti[:], mybir.ActivationFunctionType.Sigmoid
                )
                o3 = to[:].rearrange("p (g d) -> p g d", d=NODES)
                for level in range(1, DEPTH):
                    c = 2 ** (level - 1)          # number of parents
                    s0 = c - 1                    # first parent
                    s1 = 2 * c - 1                # first child
                    ch = o3[:, :, s1 : s1 + 2 * c].rearrange(
                        "p g (c two) -> p g c two", two=2
                    )
                    pa = (
                        o3[:, :, s0 : s0 + c]
                        .unsqueeze(3)
                        .broadcast_to([128, G, c, 2])
                    )
                    nc.vector.tensor_tensor(
                        out=ch, in0=ch, in1=pa, op=mybir.AluOpType.min
                    )
                # out-DMA on the ACT HWDGE ring: keeps the SP ring free for
                # in-DMAs so an out-wait can't stall in-descriptor generation
                nc.scalar.dma_start(of[:, t * W : (t + 1) * W], to[:])
    nc.compile()
    return nc


def run(x, trace=False):
    x = np.asarray(x, dtype=np.float32)
    assert x.shape == (BATCH, NODES)
    if "nc" not in _cache:
        _cache["nc"] = _build()
    nc = _cache["nc"]
    in_maps = [
        {"x": np.ascontiguousarray(x[c * B_CORE : (c + 1) * B_CORE])}
        for c in range(NCORES)
    ]
    res = run_bass_kernel_spmd(nc, in_maps, list(range(NCORES)), trace=trace)
    out = np.concatenate([res.results[c]["out"] for c in range(NCORES)], axis=0)
    return out, res


def kernel(x):
    out, _ = run(x)
    return out
